# revision 1
# baseline (speedup 1.0000x reference)
"""Trainium2 Bass kernel for ContentPopularityJointAttention.

Computes, for each batch row b:
    mp     = concat(m[b], p[b])            # (50, 512)
    hidden = tanh(mp @ Wu)                 # (50, 512)
    s      = hidden @ bvec                 # (50,)
    u[b]   = (sum_n s_n * m[b,n]) / (sum_n s_n)   # (256,)

Sharding: pure data parallel over the batch dim across 8 NeuronCores.

Per-core dataflow (tokens = batch-rows*50 = 25600, processed in 128-token
chunks):
  1. DMA m,p chunk into one [128, 512] fp32 SBUF tile (token-major).
  2. 4 PE transposes -> PSUM [128(d), 512(tok-chunked)] fp32  (feature-major).
  3. fp16 hi/lo split of the transposed data (precision: the sum-normalized
     attention amplifies error ~1/|sum s|, so the hidden matmul needs
     ~fp32-grade products; a 3-term fp16 split reaches 4e-4 rel error
     at 3 cycles/row instead of fp32's 4).
  4. 12 fp16 matmuls: (hiT@Wu_hi + loT@Wu_hi + hiT@Wu_lo), Wu moving,
     mpT chunks stationary -> hidden [128(tok), 512] fp32 PSUM.
  5. ACT tanh -> SBUF fp32.
  6. DVE tensor_tensor_reduce with b replicated across partitions ->
     per-token scores s [128, 1] fp32 (products in fp32).
  7. DVE s * block-diagonal row mask -> lhsT [128, 68] fp16; one fp16
     pooling matmul with rhs = [m | ones] accumulates [sum s*m | sum s]
     into a 64-row group PSUM (rows of 50 tokens may straddle chunks;
     PSUM accumulation handles the overlap).
  8. Per 64-row group: DVE reciprocal + scale -> u rows, DMA out.
"""

import numpy as np
from contextlib import ExitStack

import concourse.bass as bass
import concourse.bacc as bacc
import concourse.tile as tile
from concourse import mybir
from concourse.bass_utils import run_bass_kernel_spmd

N_CORES = 8
B_FULL, N_TOK, MD, PD = 4096, 50, 256, 256
D = MD + PD          # 512 contraction dim
K = 512              # hidden dim
CHUNK = 128          # tokens per chunk (partition dim)
GROUP_ROWS = 64      # batch rows per pooling PSUM accumulation group
GROUP_CHUNKS = GROUP_ROWS * N_TOK // CHUNK   # 25
POOL_P = 68          # pooling PSUM partitions (max local row 63 + span 4)

f32 = mybir.dt.float32
f16 = mybir.dt.float16
bf16 = mybir.dt.bfloat16


def build_program(b_shard: int):
    """Build the single-core Bass program (SPMD: same program, all cores)."""
    tokens = b_shard * N_TOK
    assert tokens % (CHUNK * GROUP_CHUNKS) == 0
    n_groups = b_shard // GROUP_ROWS

    nc = bacc.Bacc("TRN2", target_bir_lowering=False, debug=False,
                   num_devices=N_CORES)

    m_d = nc.dram_tensor("m", [tokens, MD], f32, kind="ExternalInput").ap()
    p_d = nc.dram_tensor("p", [tokens, PD], f32, kind="ExternalInput").ap()
    wu_hi_d = nc.dram_tensor("wu_hi", [128, 4, K], f16, kind="ExternalInput").ap()
    wu_lo_d = nc.dram_tensor("wu_lo", [128, 4, K], f16, kind="ExternalInput").ap()
    brep_d = nc.dram_tensor("brep", [128, K], f32, kind="ExternalInput").ap()
    ident_d = nc.dram_tensor("ident", [128, 128], f32, kind="ExternalInput").ap()
    masks_d = nc.dram_tensor("masks", [128, GROUP_CHUNKS, POOL_P], f16,
                             kind="ExternalInput").ap()
    u_d = nc.dram_tensor("u", [b_shard, MD], f32, kind="ExternalOutput").ap()

    with tile.TileContext(nc) as tc, ExitStack() as ctx:
        singles = ctx.enter_context(tc.tile_pool(name="singles", bufs=1))
        io_pool = ctx.enter_context(tc.tile_pool(name="io", bufs=4))
        work = ctx.enter_context(tc.tile_pool(name="work", bufs=3))
        psum_t = ctx.enter_context(tc.tile_pool(name="psumT", bufs=2, space="PSUM"))
        psum_h = ctx.enter_context(tc.tile_pool(name="psumH", bufs=2, space="PSUM"))
        psum_u = ctx.enter_context(tc.tile_pool(name="psumU", bufs=2, space="PSUM"))

        wu_hi_sb = singles.tile([128, 4, K], f16)
        nc.gpsimd.dma_start(out=wu_hi_sb[:], in_=wu_hi_d)
        wu_lo_sb = singles.tile([128, 4, K], f16)
        nc.gpsimd.dma_start(out=wu_lo_sb[:], in_=wu_lo_d)
        brep_sb = singles.tile([128, K], f32)
        nc.gpsimd.dma_start(out=brep_sb[:], in_=brep_d)
        ident_sb = singles.tile([128, 128], f32)
        nc.gpsimd.dma_start(out=ident_sb[:], in_=ident_d)
        masks_sb = singles.tile([128, GROUP_CHUNKS, POOL_P], f16)
        nc.gpsimd.dma_start(out=masks_sb[:], in_=masks_d)

        for g in range(n_groups):
            pool_ps = psum_u.tile([POOL_P, MD + 1], f32)
            for l in range(GROUP_CHUNKS):
                c = g * GROUP_CHUNKS + l
                t0 = c * CHUNK

                mp32 = io_pool.tile([128, D], f32)
                nc.gpsimd.dma_start(out=mp32[:, 0:MD], in_=m_d[t0:t0 + CHUNK, :])
                nc.gpsimd.dma_start(out=mp32[:, MD:D], in_=p_d[t0:t0 + CHUNK, :])

                # transpose to feature-major
                psT = psum_t.tile([128, D], f32)
                for j in range(4):
                    nc.tensor.transpose(
                        psT[:, j * 128:(j + 1) * 128],
                        mp32[:, j * 128:(j + 1) * 128],
                        ident_sb[:],
                    )

                # fp16 hi/lo split (in transposed domain)
                mpT_hi = work.tile([128, D], f16)
                nc.scalar.copy(out=mpT_hi[:], in_=psT[:])
                mpT_hi32 = work.tile([128, D], f32)
                nc.gpsimd.tensor_copy(out=mpT_hi32[:], in_=mpT_hi[:])
                mpT_lo = work.tile([128, D], f16)
                nc.vector.tensor_sub(mpT_lo[:], psT[:], mpT_hi32[:])

                # hidden = tanh(mp @ Wu), 3-term fp16 split
                hid = psum_h.tile([128, K], f32)
                n_mm = 12
                i_mm = 0
                for lhs in (mpT_hi, mpT_lo):
                    for j in range(4):
                        nc.tensor.matmul(
                            hid[:],
                            lhsT=lhs[:, j * 128:(j + 1) * 128],
                            rhs=wu_hi_sb[:, j, :],
                            start=(i_mm == 0),
                            stop=(i_mm == n_mm - 1),
                        )
                        i_mm += 1
                for j in range(4):
                    nc.tensor.matmul(
                        hid[:],
                        lhsT=mpT_hi[:, j * 128:(j + 1) * 128],
                        rhs=wu_lo_sb[:, j, :],
                        start=(i_mm == 0),
                        stop=(i_mm == n_mm - 1),
                    )
                    i_mm += 1

                tanhH = work.tile([128, K], f32)
                nc.scalar.activation(out=tanhH[:], in_=hid[:],
                                     func=mybir.ActivationFunctionType.Tanh)

                # s[tok] = sum_k tanhH * b   (fp32 products on DVE;
                # tensor_tensor_reduce crashes NRT in this env, use two ops)
                scr = work.tile([128, K], f32)
                s = work.tile([128, 1], f32)
                nc.vector.tensor_mul(scr[:], tanhH[:], brep_sb[:])
                nc.vector.reduce_sum(s[:], scr[:], axis=mybir.AxisListType.X)

                # block-diagonal pooling lhsT and [m | 1] rhs (fp32: the
                # ones-column sum S is cancellation-amplified, fp16 is not
                # enough there)
                blk = work.tile([128, POOL_P], f32)
                nc.vector.tensor_scalar_mul(blk[:], masks_sb[:, l, :], s[:])
                m16 = work.tile([128, MD + 1], f32)
                nc.gpsimd.tensor_copy(out=m16[:, 0:MD], in_=mp32[:, 0:MD])
                nc.vector.memset(m16[:, MD:MD + 1], 1.0)
                nc.tensor.matmul(
                    pool_ps[:],
                    lhsT=blk[:],
                    rhs=m16[:],
                    start=(l == 0),
                    stop=(l == GROUP_CHUNKS - 1),
                )

            rS = work.tile([GROUP_ROWS, 1], f32)
            nc.vector.reciprocal(rS[:], pool_ps[0:GROUP_ROWS, MD:MD + 1])
            u_sb = io_pool.tile([GROUP_ROWS, MD], f32)
            nc.vector.tensor_scalar_mul(u_sb[:], pool_ps[0:GROUP_ROWS, 0:MD], rS[:])
            nc.gpsimd.dma_start(
                out=u_d[g * GROUP_ROWS:(g + 1) * GROUP_ROWS, :], in_=u_sb[:])

    nc.compile()
    return nc


def host_constants(Wu: np.ndarray, b: np.ndarray):
    Wu = np.asarray(Wu, np.float32)
    b = np.asarray(b, np.float32)
    wu_hi16 = Wu.astype(np.float16)
    wu_lo16 = (Wu - wu_hi16.astype(np.float32)).astype(np.float16)
    # [d, k] -> [d%128, d//128, k]
    wu_hi = np.ascontiguousarray(wu_hi16.reshape(4, 128, K).transpose(1, 0, 2))
    wu_lo = np.ascontiguousarray(wu_lo16.reshape(4, 128, K).transpose(1, 0, 2))
    brep = np.ascontiguousarray(np.broadcast_to(b, (128, K)))
    ident = np.eye(128, dtype=np.float32)
    tp = np.arange(128)[:, None, None]
    ll = np.arange(GROUP_CHUNKS)[None, :, None]
    rr = np.arange(POOL_P)[None, None, :]
    masks = (((CHUNK * ll + tp) // N_TOK) == rr).astype(np.float16)
    return {"wu_hi": wu_hi, "wu_lo": wu_lo, "brep": brep, "ident": ident,
            "masks": masks}


_prog_cache: dict = {}


def get_program(b_shard: int):
    if b_shard not in _prog_cache:
        _prog_cache[b_shard] = build_program(b_shard)
    return _prog_cache[b_shard]


def kernel(m: np.ndarray, p: np.ndarray, Wu: np.ndarray, b: np.ndarray
           ) -> np.ndarray:
    m = np.ascontiguousarray(np.asarray(m, np.float32))
    p = np.ascontiguousarray(np.asarray(p, np.float32))
    B = m.shape[0]
    assert B % N_CORES == 0
    b_shard = B // N_CORES

    nc = get_program(b_shard)
    consts = host_constants(Wu, b)

    mf = m.reshape(B * N_TOK, MD)
    pf = p.reshape(B * N_TOK, PD)
    tok_sh = b_shard * N_TOK
    in_maps = []
    for c in range(N_CORES):
        in_maps.append({
            "m": mf[c * tok_sh:(c + 1) * tok_sh],
            "p": pf[c * tok_sh:(c + 1) * tok_sh],
            **consts,
        })
    res = run_bass_kernel_spmd(nc, in_maps, list(range(N_CORES)))
    u = np.concatenate([res.results[c]["u"] for c in range(N_CORES)], axis=0)
    return u.astype(np.float32)



# revision 2
# speedup vs baseline: 1.2994x; 1.2994x over previous
"""Trainium2 Bass kernel for ContentPopularityJointAttention.

Computes, for each batch row b:
    mp     = concat(m[b], p[b])            # (50, 512)
    hidden = tanh(mp @ Wu)                 # (50, 512)
    s      = hidden @ bvec                 # (50,)
    u[b]   = (sum_n s_n * m[b,n]) / (sum_n s_n)   # (256,)

Sharding: pure data parallel over the batch dim across 8 NeuronCores.

Precision notes (measured): the sum-normalized attention amplifies score
errors by ~1/|sum s|; the hidden matmul needs >=16 valid mantissa bits on
BOTH operands (fp16 1-term: 0.39 rel err, fp32r single-pass HW matmul:
~1.5e-4 products -> ~0.2 rel err, both FAIL). A 3-term fp16 hi/lo split
(xh@Wh + xl@Wh + xh@Wl) gives 6.8e-4. The pooling NUMERATOR tolerates
fp16 (2.9e-4) but the ones-column S (denominator) must be true fp32.

Per-core dataflow (tokens = rows*50, 128-token chunks; PE is the
bottleneck at 6404 cycles/chunk, everything else hides under it):
  1. Host pre-splits x=concat(m,p) into fp16 hi/lo and pre-transposes to
     feature-major chunk-blocked layout mpT [128,C,4(dchunk),2(hi/lo),128]
     (one 2KB-descriptor DMA per chunk; no PE transposes, no PSUM->SBUF
     bounce). Token-major m_hi [tok,256] f16 is DMA'd for pooling rhs.
  2. 12 fp16 matmuls (3-term split, Wu moving, ap=512) -> hid PSUM f32.
  3. ACT tanh -> SBUF f32.
  4. DVE mul by b-replicated (fp32 products) + reduce -> s [128,1] f32.
  5. DVE s * block-diag row mask -> blk32 f32; Pool copy -> blk16 f16.
  6. PE pooling: blk16 @ m_hi -> pool_m PSUM [68,256] (fp16, 256c) and
     blk32 @ ones -> pool_s PSUM [68,1] (fp32, 4c), accumulated over the
     25 chunks of each 64-row group.
  7. Group end: DVE reciprocal + scale -> u rows, DMA out.
"""

import numpy as np
from contextlib import ExitStack

import concourse.bass as bass
import concourse.bacc as bacc
import concourse.tile as tile
from concourse import mybir
from concourse.bass_utils import run_bass_kernel_spmd

N_CORES = 8
B_FULL, N_TOK, MD, PD = 4096, 50, 256, 256
D = MD + PD          # 512 contraction dim
K = 512              # hidden dim
CHUNK = 128          # tokens per chunk (partition dim)
GROUP_ROWS = 64      # batch rows per pooling PSUM accumulation group
GROUP_CHUNKS = GROUP_ROWS * N_TOK // CHUNK   # 25
POOL_P = 68          # pooling PSUM partitions (max local row 63 + span 4)

f32 = mybir.dt.float32
f16 = mybir.dt.float16


def build_program(b_shard: int):
    """Build the single-core Bass program (SPMD: same program, all cores)."""
    tokens = b_shard * N_TOK
    assert tokens % (CHUNK * GROUP_CHUNKS) == 0
    n_groups = b_shard // GROUP_ROWS
    n_chunks = tokens // CHUNK

    nc = bacc.Bacc("TRN2", target_bir_lowering=False, debug=False,
                   num_devices=N_CORES)

    # feature-major fp16 hi/lo of concat(m,p), chunk-blocked:
    # mpT[q, c, j, h, t] = x_h[c*128+t, j*128+q]
    mpT_d = nc.dram_tensor("mpT", [128, n_chunks, 4, 2, CHUNK], f16,
                           kind="ExternalInput").ap()
    # token-major fp16(m) for the pooling rhs
    mhi_d = nc.dram_tensor("mhi", [tokens, MD], f16, kind="ExternalInput").ap()
    wu_hi_d = nc.dram_tensor("wu_hi", [128, 4, K], f16, kind="ExternalInput").ap()
    wu_lo_d = nc.dram_tensor("wu_lo", [128, 4, K], f16, kind="ExternalInput").ap()
    brep_d = nc.dram_tensor("brep", [128, K], f32, kind="ExternalInput").ap()
    masks_d = nc.dram_tensor("masks", [128, GROUP_CHUNKS, POOL_P], f32,
                             kind="ExternalInput").ap()
    ones_d = nc.dram_tensor("ones", [128, 1], f32, kind="ExternalInput").ap()
    u_d = nc.dram_tensor("u", [b_shard, MD], f32, kind="ExternalOutput").ap()

    with tile.TileContext(nc) as tc, ExitStack() as ctx:
        singles = ctx.enter_context(tc.tile_pool(name="singles", bufs=1))
        io_x = ctx.enter_context(tc.tile_pool(name="iox", bufs=3))
        io_m = ctx.enter_context(tc.tile_pool(name="iom", bufs=3))
        io_u = ctx.enter_context(tc.tile_pool(name="iou", bufs=2))
        work = ctx.enter_context(tc.tile_pool(name="work", bufs=3))
        psum_h = ctx.enter_context(tc.tile_pool(name="psumH", bufs=2, space="PSUM"))
        psum_m = ctx.enter_context(tc.tile_pool(name="psumM", bufs=2, space="PSUM"))
        psum_s = ctx.enter_context(tc.tile_pool(name="psumS", bufs=2, space="PSUM"))

        wu_hi_sb = singles.tile([128, 4, K], f16)
        nc.gpsimd.dma_start(out=wu_hi_sb[:], in_=wu_hi_d)
        wu_lo_sb = singles.tile([128, 4, K], f16)
        nc.gpsimd.dma_start(out=wu_lo_sb[:], in_=wu_lo_d)
        brep_sb = singles.tile([128, K], f32)
        nc.gpsimd.dma_start(out=brep_sb[:], in_=brep_d)
        masks_sb = singles.tile([128, GROUP_CHUNKS, POOL_P], f32)
        nc.gpsimd.dma_start(out=masks_sb[:], in_=masks_d)
        ones_sb = singles.tile([128, 1], f32)
        nc.gpsimd.dma_start(out=ones_sb[:], in_=ones_d)

        for g in range(n_groups):
            pool_m = psum_m.tile([POOL_P, MD], f32)
            pool_s = psum_s.tile([POOL_P, 1], f32)
            for l in range(GROUP_CHUNKS):
                c = g * GROUP_CHUNKS + l
                t0 = c * CHUNK

                xT = io_x.tile([128, 4, 2, CHUNK], f16)
                nc.sync.dma_start(out=xT[:], in_=mpT_d[:, c])
                mh = io_m.tile([128, MD], f16)
                nc.scalar.dma_start(out=mh[:], in_=mhi_d[t0:t0 + CHUNK, :])

                # hidden = tanh(mp @ Wu), 3-term fp16 split
                hid = psum_h.tile([128, K], f32)
                i_mm = 0
                for h_x, wu_sb in ((0, wu_hi_sb), (1, wu_hi_sb), (0, wu_lo_sb)):
                    for j in range(4):
                        nc.tensor.matmul(
                            hid[:],
                            lhsT=xT[:, j, h_x, :],
                            rhs=wu_sb[:, j, :],
                            start=(i_mm == 0),
                            stop=(i_mm == 11),
                        )
                        i_mm += 1

                tanhH = work.tile([128, K], f32)
                nc.scalar.activation(out=tanhH[:], in_=hid[:],
                                     func=mybir.ActivationFunctionType.Tanh)

                # s[tok] = sum_k tanhH * b   (fp32 products on DVE)
                scr = work.tile([128, K], f32)
                s = work.tile([128, 1], f32)
                nc.vector.tensor_mul(scr[:], tanhH[:], brep_sb[:])
                nc.vector.reduce_sum(s[:], scr[:], axis=mybir.AxisListType.X)

                # block-diagonal pooling lhsT: fp32 for the S column
                # (cancellation-amplified), fp16 for the m columns
                blk32 = work.tile([128, POOL_P], f32)
                nc.vector.tensor_scalar_mul(blk32[:], masks_sb[:, l, :], s[:])
                blk16 = work.tile([128, POOL_P], f16)
                nc.gpsimd.tensor_copy(out=blk16[:], in_=blk32[:])

                nc.tensor.matmul(
                    pool_m[:],
                    lhsT=blk16[:],
                    rhs=mh[:],
                    start=(l == 0),
                    stop=(l == GROUP_CHUNKS - 1),
                )
                nc.tensor.matmul(
                    pool_s[:],
                    lhsT=blk32[:],
                    rhs=ones_sb[:],
                    start=(l == 0),
                    stop=(l == GROUP_CHUNKS - 1),
                )

            rS = work.tile([GROUP_ROWS, 1], f32)
            nc.vector.reciprocal(rS[:], pool_s[0:GROUP_ROWS, :])
            u_sb = io_u.tile([GROUP_ROWS, MD], f32)
            nc.vector.tensor_scalar_mul(u_sb[:], pool_m[0:GROUP_ROWS, 0:MD], rS[:])
            nc.sync.dma_start(
                out=u_d[g * GROUP_ROWS:(g + 1) * GROUP_ROWS, :], in_=u_sb[:])

    nc.compile()
    return nc


def host_constants(Wu: np.ndarray, b: np.ndarray):
    Wu = np.asarray(Wu, np.float32)
    b = np.asarray(b, np.float32)
    wu_hi16 = Wu.astype(np.float16)
    wu_lo16 = (Wu - wu_hi16.astype(np.float32)).astype(np.float16)
    # [d, k] -> [d%128, d//128, k]
    wu_hi = np.ascontiguousarray(wu_hi16.reshape(4, 128, K).transpose(1, 0, 2))
    wu_lo = np.ascontiguousarray(wu_lo16.reshape(4, 128, K).transpose(1, 0, 2))
    brep = np.ascontiguousarray(np.broadcast_to(b, (128, K)))
    tp = np.arange(128)[:, None, None]
    ll = np.arange(GROUP_CHUNKS)[None, :, None]
    rr = np.arange(POOL_P)[None, None, :]
    masks = (((CHUNK * ll + tp) // N_TOK) == rr).astype(np.float32)
    ones = np.ones((128, 1), np.float32)
    return {"wu_hi": wu_hi, "wu_lo": wu_lo, "brep": brep, "masks": masks,
            "ones": ones}


def host_shard_inputs(m_shard: np.ndarray, p_shard: np.ndarray):
    """Per-shard data tensors: fp16 hi/lo feature-major chunk-blocked mpT
    and token-major fp16 m for the pooling rhs."""
    tokens = m_shard.shape[0] * N_TOK
    n_chunks = tokens // CHUNK
    x = np.concatenate(
        [m_shard.reshape(tokens, MD), p_shard.reshape(tokens, PD)], axis=1)
    xh = x.astype(np.float16)
    xl = (x - xh.astype(np.float32)).astype(np.float16)
    # [tok, 512] -> [128q, n_chunks, 4j, 128t]
    def to_fmajor(a):
        return a.reshape(n_chunks, CHUNK, 4, 128).transpose(3, 0, 2, 1)
    mpT = np.ascontiguousarray(
        np.stack([to_fmajor(xh), to_fmajor(xl)], axis=3))
    mhi = np.ascontiguousarray(xh[:, 0:MD])
    return {"mpT": mpT, "mhi": mhi}


_prog_cache: dict = {}


def get_program(b_shard: int):
    if b_shard not in _prog_cache:
        _prog_cache[b_shard] = build_program(b_shard)
    return _prog_cache[b_shard]


def kernel(m: np.ndarray, p: np.ndarray, Wu: np.ndarray, b: np.ndarray
           ) -> np.ndarray:
    m = np.ascontiguousarray(np.asarray(m, np.float32))
    p = np.ascontiguousarray(np.asarray(p, np.float32))
    B = m.shape[0]
    assert B % N_CORES == 0
    b_shard = B // N_CORES

    nc = get_program(b_shard)
    consts = host_constants(Wu, b)

    in_maps = []
    for c in range(N_CORES):
        ms = m[c * b_shard:(c + 1) * b_shard]
        ps = p[c * b_shard:(c + 1) * b_shard]
        in_maps.append({**host_shard_inputs(ms, ps), **consts})
    res = run_bass_kernel_spmd(nc, in_maps, list(range(N_CORES)))
    u = np.concatenate([res.results[c]["u"] for c in range(N_CORES)], axis=0)
    return u.astype(np.float32)


# revision 3
# speedup vs baseline: 1.3251x; 1.0198x over previous
"""Trainium2 Bass kernel for ContentPopularityJointAttention.

Computes, for each batch row b:
    mp     = concat(m[b], p[b])            # (50, 512)
    hidden = tanh(mp @ Wu)                 # (50, 512)
    s      = hidden @ bvec                 # (50,)
    u[b]   = (sum_n s_n * m[b,n]) / (sum_n s_n)   # (256,)

Sharding: pure data parallel over the batch dim across 8 NeuronCores.

Precision notes (measured): the sum-normalized attention amplifies score
errors by ~1/|sum s|; the hidden matmul needs >=16 valid mantissa bits on
BOTH operands (fp16 1-term: 0.39 rel err; fp32r single-pass HW matmul has
~1.5e-4 product error -> ~0.2 rel err; both FAIL the 2e-2 gate). A 3-term
fp16 hi/lo split (xh@Wh + xl@Wh + xh@Wl) gives 6.8e-4. The pooling
NUMERATOR tolerates fp16 (2.9e-4) but the ones-column S (denominator)
must be true fp32.

Per-core dataflow (tokens = rows*50, 128-token chunks; PE is the
bottleneck at ~6276 cycles/chunk, all other engines hide under it):
  1. Host pre-splits x=concat(m,p) into fp16 hi/lo and pre-transposes to
     feature-major chunk-blocked layout mpT [128,C,4(dchunk),2(hi/lo),128]
     (one 2KB-per-partition-descriptor DMA per chunk; no PE transposes).
     Token-major m_hi [tok,256] f16 is DMA'd for the pooling stationary.
  2. 12 fp16 matmuls (3-term split, Wu moving, ap=512) -> hid PSUM f32.
  3. ACT tanh -> SBUF f32.
  4. DVE mul by b-replicated (fp32 products) + reduce -> s [128,1] f32.
  5. DVE s * block-diag row mask -> blk32 f32; Pool copy -> blk16 f16.
  6. PE pooling, flipped so the small mask side streams: two matmuls
     lhsT=mh half [128t,128d] (stationary), rhs=blk16 [128t,64r] fp16
     (ap=64 -> 64c each) -> uT PSUM [128d,64r] per d-half, plus
     lhsT=blk32 @ rhs=ones (fp32, ap=1 -> 4c) -> S PSUM [64,1],
     all accumulated over the 25 chunks of each 64-row group.
  7. Group end: ACT copies uT/S PSUM->SBUF, DMA out. The final
     u = uT.T / S normalization happens on the host during unshard
     (exact fp32 divide, zero device cost).
"""

import numpy as np
from contextlib import ExitStack

import concourse.bass as bass
import concourse.bacc as bacc
import concourse.tile as tile
from concourse import mybir
from concourse.bass_utils import run_bass_kernel_spmd

N_CORES = 8
B_FULL, N_TOK, MD, PD = 4096, 50, 256, 256
D = MD + PD          # 512 contraction dim
K = 512              # hidden dim
CHUNK = 128          # tokens per chunk (partition dim)
GROUP_ROWS = 64      # batch rows per pooling PSUM accumulation group
GROUP_CHUNKS = GROUP_ROWS * N_TOK // CHUNK   # 25
POOL_P = 64          # pooling free dim (rows per group; max local row 63)

f32 = mybir.dt.float32
f16 = mybir.dt.float16


def build_program(b_shard: int):
    """Build the single-core Bass program (SPMD: same program, all cores)."""
    tokens = b_shard * N_TOK
    assert tokens % (CHUNK * GROUP_CHUNKS) == 0
    n_groups = b_shard // GROUP_ROWS
    n_chunks = tokens // CHUNK

    nc = bacc.Bacc("TRN2", target_bir_lowering=False, debug=False,
                   num_devices=N_CORES)

    # feature-major fp16 hi/lo of concat(m,p), chunk-blocked:
    # mpT[q, c, j, h, t] = x_h[c*128+t, j*128+q]
    mpT_d = nc.dram_tensor("mpT", [128, n_chunks, 4, 2, CHUNK], f16,
                           kind="ExternalInput").ap()
    # token-major fp16(m) for the pooling stationary operand
    mhi_d = nc.dram_tensor("mhi", [tokens, MD], f16, kind="ExternalInput").ap()
    wu_hi_d = nc.dram_tensor("wu_hi", [128, 4, K], f16, kind="ExternalInput").ap()
    wu_lo_d = nc.dram_tensor("wu_lo", [128, 4, K], f16, kind="ExternalInput").ap()
    brep_d = nc.dram_tensor("brep", [128, K], f32, kind="ExternalInput").ap()
    masks_d = nc.dram_tensor("masks", [128, GROUP_CHUNKS, POOL_P], f32,
                             kind="ExternalInput").ap()
    ones_d = nc.dram_tensor("ones", [128, 1], f32, kind="ExternalInput").ap()
    # transposed pooled output + per-row score sums (host divides)
    uT_d = nc.dram_tensor("uT", [n_groups, 128, 2, POOL_P], f32,
                          kind="ExternalOutput").ap()
    sS_d = nc.dram_tensor("sS", [n_groups, POOL_P, 1], f32,
                          kind="ExternalOutput").ap()

    with tile.TileContext(nc) as tc, ExitStack() as ctx:
        singles = ctx.enter_context(tc.tile_pool(name="singles", bufs=1))
        io_x = ctx.enter_context(tc.tile_pool(name="iox", bufs=3))
        io_m = ctx.enter_context(tc.tile_pool(name="iom", bufs=3))
        io_u = ctx.enter_context(tc.tile_pool(name="iou", bufs=2))
        work = ctx.enter_context(tc.tile_pool(name="work", bufs=3))
        psum_h = ctx.enter_context(tc.tile_pool(name="psumH", bufs=2, space="PSUM"))
        psum_a = ctx.enter_context(tc.tile_pool(name="psumA", bufs=2, space="PSUM"))
        psum_b = ctx.enter_context(tc.tile_pool(name="psumB", bufs=2, space="PSUM"))
        psum_s = ctx.enter_context(tc.tile_pool(name="psumS", bufs=2, space="PSUM"))

        wu_hi_sb = singles.tile([128, 4, K], f16)
        nc.gpsimd.dma_start(out=wu_hi_sb[:], in_=wu_hi_d)
        wu_lo_sb = singles.tile([128, 4, K], f16)
        nc.gpsimd.dma_start(out=wu_lo_sb[:], in_=wu_lo_d)
        brep_sb = singles.tile([128, K], f32)
        nc.gpsimd.dma_start(out=brep_sb[:], in_=brep_d)
        masks_sb = singles.tile([128, GROUP_CHUNKS, POOL_P], f32)
        nc.gpsimd.dma_start(out=masks_sb[:], in_=masks_d)
        ones_sb = singles.tile([128, 1], f32)
        nc.gpsimd.dma_start(out=ones_sb[:], in_=ones_d)

        for g in range(n_groups):
            pool_a = psum_a.tile([128, POOL_P], f32)   # d 0..127
            pool_b = psum_b.tile([128, POOL_P], f32)   # d 128..255
            pool_s = psum_s.tile([POOL_P, 1], f32)
            for l in range(GROUP_CHUNKS):
                c = g * GROUP_CHUNKS + l
                t0 = c * CHUNK

                xT = io_x.tile([128, 4, 2, CHUNK], f16)
                nc.sync.dma_start(out=xT[:], in_=mpT_d[:, c])
                mh = io_m.tile([128, MD], f16)
                nc.scalar.dma_start(out=mh[:], in_=mhi_d[t0:t0 + CHUNK, :])

                # hidden = tanh(mp @ Wu), 3-term fp16 split
                hid = psum_h.tile([128, K], f32)
                i_mm = 0
                for h_x, wu_sb in ((0, wu_hi_sb), (1, wu_hi_sb), (0, wu_lo_sb)):
                    for j in range(4):
                        nc.tensor.matmul(
                            hid[:],
                            lhsT=xT[:, j, h_x, :],
                            rhs=wu_sb[:, j, :],
                            start=(i_mm == 0),
                            stop=(i_mm == 11),
                        )
                        i_mm += 1

                tanhH = work.tile([128, K], f32)
                nc.scalar.activation(out=tanhH[:], in_=hid[:],
                                     func=mybir.ActivationFunctionType.Tanh)

                # s[tok] = sum_k tanhH * b   (fp32 products on DVE)
                scr = work.tile([128, K], f32)
                s = work.tile([128, 1], f32)
                nc.vector.tensor_mul(scr[:], tanhH[:], brep_sb[:])
                nc.vector.reduce_sum(s[:], scr[:], axis=mybir.AxisListType.X)

                # block-diagonal pooling masks: fp32 for the S column
                # (cancellation-amplified), fp16 for the m pooling
                blk32 = work.tile([128, POOL_P], f32)
                nc.vector.tensor_scalar_mul(blk32[:], masks_sb[:, l, :], s[:])
                blk16 = work.tile([128, POOL_P], f16)
                nc.gpsimd.tensor_copy(out=blk16[:], in_=blk32[:])

                nc.tensor.matmul(
                    pool_a[:],
                    lhsT=mh[:, 0:128],
                    rhs=blk16[:],
                    start=(l == 0),
                    stop=(l == GROUP_CHUNKS - 1),
                )
                nc.tensor.matmul(
                    pool_b[:],
                    lhsT=mh[:, 128:256],
                    rhs=blk16[:],
                    start=(l == 0),
                    stop=(l == GROUP_CHUNKS - 1),
                )
                nc.tensor.matmul(
                    pool_s[:],
                    lhsT=blk32[:],
                    rhs=ones_sb[:],
                    start=(l == 0),
                    stop=(l == GROUP_CHUNKS - 1),
                )

            u_sb = io_u.tile([128, 2, POOL_P], f32)
            nc.scalar.copy(out=u_sb[:, 0, :], in_=pool_a[:])
            nc.scalar.copy(out=u_sb[:, 1, :], in_=pool_b[:])
            s_sb = io_u.tile([POOL_P, 1], f32)
            nc.vector.tensor_copy(out=s_sb[:], in_=pool_s[:])
            nc.sync.dma_start(out=uT_d[g], in_=u_sb[:])
            nc.sync.dma_start(out=sS_d[g], in_=s_sb[:])

    nc.compile()
    return nc


def host_constants(Wu: np.ndarray, b: np.ndarray):
    Wu = np.asarray(Wu, np.float32)
    b = np.asarray(b, np.float32)
    wu_hi16 = Wu.astype(np.float16)
    wu_lo16 = (Wu - wu_hi16.astype(np.float32)).astype(np.float16)
    # [d, k] -> [d%128, d//128, k]
    wu_hi = np.ascontiguousarray(wu_hi16.reshape(4, 128, K).transpose(1, 0, 2))
    wu_lo = np.ascontiguousarray(wu_lo16.reshape(4, 128, K).transpose(1, 0, 2))
    brep = np.ascontiguousarray(np.broadcast_to(b, (128, K)))
    tp = np.arange(128)[:, None, None]
    ll = np.arange(GROUP_CHUNKS)[None, :, None]
    rr = np.arange(POOL_P)[None, None, :]
    masks = (((CHUNK * ll + tp) // N_TOK) == rr).astype(np.float32)
    ones = np.ones((128, 1), np.float32)
    return {"wu_hi": wu_hi, "wu_lo": wu_lo, "brep": brep, "masks": masks,
            "ones": ones}


def host_shard_inputs(m_shard: np.ndarray, p_shard: np.ndarray):
    """Per-shard data tensors: fp16 hi/lo feature-major chunk-blocked mpT
    and token-major fp16 m for the pooling stationary operand."""
    tokens = m_shard.shape[0] * N_TOK
    n_chunks = tokens // CHUNK
    x = np.concatenate(
        [m_shard.reshape(tokens, MD), p_shard.reshape(tokens, PD)], axis=1)
    xh = x.astype(np.float16)
    xl = (x - xh.astype(np.float32)).astype(np.float16)
    # [tok, 512] -> [128q, n_chunks, 4j, 128t]
    def to_fmajor(a):
        return a.reshape(n_chunks, CHUNK, 4, 128).transpose(3, 0, 2, 1)
    mpT = np.ascontiguousarray(
        np.stack([to_fmajor(xh), to_fmajor(xl)], axis=3))
    mhi = np.ascontiguousarray(xh[:, 0:MD])
    return {"mpT": mpT, "mhi": mhi}


def unshard_output(uT: np.ndarray, sS: np.ndarray) -> np.ndarray:
    """[n_groups,128,2,64] pooled sums + [n_groups,64,1] score sums ->
    normalized u [rows, 256]."""
    n_groups = uT.shape[0]
    # uT[g, q, h, r] -> u[g*64+r, h*128+q]
    u = uT.transpose(0, 3, 2, 1).reshape(n_groups * POOL_P, MD)
    S = sS.reshape(n_groups * POOL_P, 1)
    return u / S


_prog_cache: dict = {}


def get_program(b_shard: int):
    if b_shard not in _prog_cache:
        _prog_cache[b_shard] = build_program(b_shard)
    return _prog_cache[b_shard]


def kernel(m: np.ndarray, p: np.ndarray, Wu: np.ndarray, b: np.ndarray
           ) -> np.ndarray:
    m = np.ascontiguousarray(np.asarray(m, np.float32))
    p = np.ascontiguousarray(np.asarray(p, np.float32))
    B = m.shape[0]
    assert B % N_CORES == 0
    b_shard = B // N_CORES

    nc = get_program(b_shard)
    consts = host_constants(Wu, b)

    in_maps = []
    for c in range(N_CORES):
        ms = m[c * b_shard:(c + 1) * b_shard]
        ps = p[c * b_shard:(c + 1) * b_shard]
        in_maps.append({**host_shard_inputs(ms, ps), **consts})
    res = run_bass_kernel_spmd(nc, in_maps, list(range(N_CORES)))
    u = np.concatenate(
        [unshard_output(res.results[c]["uT"], res.results[c]["sS"])
         for c in range(N_CORES)], axis=0)
    return u.astype(np.float32)


# revision 7
# speedup vs baseline: 1.3252x; 1.0001x over previous
"""Trainium2 Bass kernel for ContentPopularityJointAttention.

Computes, for each batch row b:
    mp     = concat(m[b], p[b])            # (50, 512)
    hidden = tanh(mp @ Wu)                 # (50, 512)
    s      = hidden @ bvec                 # (50,)
    u[b]   = (sum_n s_n * m[b,n]) / (sum_n s_n)   # (256,)

Sharding: pure data parallel over the batch dim across 8 NeuronCores.

Precision notes (measured): the sum-normalized attention amplifies score
errors by ~1/|sum s|; the hidden matmul needs >=16 valid mantissa bits on
BOTH operands (fp16 1-term: 0.39 rel err; fp32r single-pass HW matmul has
~1.5e-4 product error -> ~0.2 rel err; both FAIL the 2e-2 gate). A 3-term
fp16 hi/lo split (xh@Wh + xl@Wh + xh@Wl) gives 6.8e-4. The pooling
NUMERATOR tolerates fp16 (2.9e-4) but the ones-column S (denominator)
must be true fp32.

Per-core dataflow (tokens = rows*50, 128-token chunks; PE is the
bottleneck at ~6276 cycles/chunk, all other engines hide under it):
  1. Host pre-splits x=concat(m,p) into fp16 hi/lo and pre-transposes to
     feature-major chunk-blocked layout mpT [128,C,4(dchunk),2(hi/lo),128]
     (one 2KB-per-partition-descriptor DMA per chunk; no PE transposes).
     Token-major m_hi [tok,256] f16 is DMA'd for the pooling stationary.
  2. 12 fp16 matmuls (3-term split, Wu moving, ap=512) -> hid PSUM f32.
  3. ACT tanh -> SBUF f32.
  4. DVE mul by b-replicated (fp32 products) + reduce -> s [128,1] f32.
  5. DVE s * block-diag row mask -> blk32 f32; Pool copy -> blk16 f16.
  6. PE pooling, flipped so the small mask side streams: two matmuls
     lhsT=mh half [128t,128d] (stationary), rhs=blk16 [128t,64r] fp16
     (ap=64 -> 64c each) -> uT PSUM [128d,64r] per d-half, plus
     lhsT=blk32 @ rhs=ones (fp32, ap=1 -> 4c) -> S PSUM [64,1],
     all accumulated over the 25 chunks of each 64-row group.
  7. Group end: ACT copies uT/S PSUM->SBUF, DMA out. The final
     u = uT.T / S normalization happens on the host during unshard
     (exact fp32 divide, zero device cost).
"""

import numpy as np
from contextlib import ExitStack

import concourse.bass as bass
import concourse.bacc as bacc
import concourse.tile as tile
from concourse import mybir
from concourse.bass_utils import run_bass_kernel_spmd

N_CORES = 8
B_FULL, N_TOK, MD, PD = 4096, 50, 256, 256
D = MD + PD          # 512 contraction dim
K = 512              # hidden dim
CHUNK = 128          # tokens per chunk (partition dim)
GROUP_ROWS = 64      # batch rows per pooling PSUM accumulation group
GROUP_CHUNKS = GROUP_ROWS * N_TOK // CHUNK   # 25
POOL_P = 64          # pooling free dim (rows per group; max local row 63)

f32 = mybir.dt.float32
f16 = mybir.dt.float16


def build_program(b_shard: int):
    """Build the single-core Bass program (SPMD: same program, all cores)."""
    tokens = b_shard * N_TOK
    assert tokens % (CHUNK * GROUP_CHUNKS) == 0
    n_groups = b_shard // GROUP_ROWS
    n_chunks = tokens // CHUNK

    nc = bacc.Bacc("TRN2", target_bir_lowering=False, debug=False,
                   num_devices=N_CORES)

    # feature-major fp16 hi/lo of concat(m,p), chunk-blocked:
    # mpT[q, c, j, h, t] = x_h[c*128+t, j*128+q]
    mpT_d = nc.dram_tensor("mpT", [128, n_chunks, 4, 2, CHUNK], f16,
                           kind="ExternalInput").ap()
    # token-major fp16(m) for the pooling stationary operand
    mhi_d = nc.dram_tensor("mhi", [tokens, MD], f16, kind="ExternalInput").ap()
    wu_hi_d = nc.dram_tensor("wu_hi", [128, 4, K], f16, kind="ExternalInput").ap()
    wu_lo_d = nc.dram_tensor("wu_lo", [128, 4, K], f16, kind="ExternalInput").ap()
    brep_d = nc.dram_tensor("brep", [128, K], f32, kind="ExternalInput").ap()
    masks_d = nc.dram_tensor("masks", [128, GROUP_CHUNKS, POOL_P], f32,
                             kind="ExternalInput").ap()
    masks16_d = nc.dram_tensor("masks16", [128, GROUP_CHUNKS, POOL_P], f16,
                               kind="ExternalInput").ap()
    ones_d = nc.dram_tensor("ones", [128, 1], f32, kind="ExternalInput").ap()
    # transposed pooled output + per-row score sums (host divides)
    uT_d = nc.dram_tensor("uT", [n_groups, 128, 2, POOL_P], f32,
                          kind="ExternalOutput").ap()
    sS_d = nc.dram_tensor("sS", [n_groups, POOL_P, 1], f32,
                          kind="ExternalOutput").ap()

    with tile.TileContext(nc) as tc, ExitStack() as ctx:
        singles = ctx.enter_context(tc.tile_pool(name="singles", bufs=1))
        io_x = ctx.enter_context(tc.tile_pool(name="iox", bufs=3))
        io_m = ctx.enter_context(tc.tile_pool(name="iom", bufs=3))
        io_u = ctx.enter_context(tc.tile_pool(name="iou", bufs=2))
        work = ctx.enter_context(tc.tile_pool(name="work", bufs=3))
        psum_h = ctx.enter_context(tc.tile_pool(name="psumH", bufs=2, space="PSUM"))
        psum_a = ctx.enter_context(tc.tile_pool(name="psumA", bufs=2, space="PSUM"))
        psum_b = ctx.enter_context(tc.tile_pool(name="psumB", bufs=2, space="PSUM"))
        psum_s = ctx.enter_context(tc.tile_pool(name="psumS", bufs=2, space="PSUM"))

        wu_hi_sb = singles.tile([128, 4, K], f16)
        nc.gpsimd.dma_start(out=wu_hi_sb[:], in_=wu_hi_d)
        wu_lo_sb = singles.tile([128, 4, K], f16)
        nc.gpsimd.dma_start(out=wu_lo_sb[:], in_=wu_lo_d)
        brep_sb = singles.tile([128, K], f32)
        nc.gpsimd.dma_start(out=brep_sb[:], in_=brep_d)
        masks_sb = singles.tile([128, GROUP_CHUNKS, POOL_P], f32)
        nc.gpsimd.dma_start(out=masks_sb[:], in_=masks_d)
        masks16_sb = singles.tile([128, GROUP_CHUNKS, POOL_P], f16)
        nc.gpsimd.dma_start(out=masks16_sb[:], in_=masks16_d)
        ones_sb = singles.tile([128, 1], f32)
        nc.gpsimd.dma_start(out=ones_sb[:], in_=ones_d)

        for g in range(n_groups):
            pool_a = psum_a.tile([128, POOL_P], f32)   # d 0..127
            pool_b = psum_b.tile([128, POOL_P], f32)   # d 128..255
            pool_s = psum_s.tile([POOL_P, 1], f32)
            for l in range(GROUP_CHUNKS):
                c = g * GROUP_CHUNKS + l
                t0 = c * CHUNK

                xT = io_x.tile([128, 4, 2, CHUNK], f16)
                nc.sync.dma_start(out=xT[:], in_=mpT_d[:, c])
                mh = io_m.tile([128, MD], f16)
                nc.scalar.dma_start(out=mh[:], in_=mhi_d[t0:t0 + CHUNK, :])

                # hidden = tanh(mp @ Wu), 3-term fp16 split
                hid = psum_h.tile([128, K], f32)
                i_mm = 0
                for h_x, wu_sb in ((0, wu_hi_sb), (1, wu_hi_sb), (0, wu_lo_sb)):
                    for j in range(4):
                        nc.tensor.matmul(
                            hid[:],
                            lhsT=xT[:, j, h_x, :],
                            rhs=wu_sb[:, j, :],
                            start=(i_mm == 0),
                            stop=(i_mm == 11),
                        )
                        i_mm += 1

                tanhH = work.tile([128, K], f32)
                nc.scalar.activation(out=tanhH[:], in_=hid[:],
                                     func=mybir.ActivationFunctionType.Tanh)

                # s[tok] = sum_k tanhH * b   (fp32 products on DVE)
                scr = work.tile([128, K], f32)
                s = work.tile([128, 1], f32)
                nc.vector.tensor_mul(scr[:], tanhH[:], brep_sb[:])
                nc.vector.reduce_sum(s[:], scr[:], axis=mybir.AxisListType.X)

                # block-diagonal pooling masks: fp32 for the S column
                # (cancellation-amplified), fp16 for the m pooling
                blk32 = work.tile([128, POOL_P], f32)
                nc.vector.tensor_scalar_mul(blk32[:], masks_sb[:, l, :], s[:])
                blk16 = work.tile([128, POOL_P], f16)
                nc.vector.tensor_scalar_mul(blk16[:], masks16_sb[:, l, :], s[:])

                nc.tensor.matmul(
                    pool_a[:],
                    lhsT=mh[:, 0:128],
                    rhs=blk16[:],
                    start=(l == 0),
                    stop=(l == GROUP_CHUNKS - 1),
                )
                nc.tensor.matmul(
                    pool_b[:],
                    lhsT=mh[:, 128:256],
                    rhs=blk16[:],
                    start=(l == 0),
                    stop=(l == GROUP_CHUNKS - 1),
                )
                nc.tensor.matmul(
                    pool_s[:],
                    lhsT=blk32[:],
                    rhs=ones_sb[:],
                    start=(l == 0),
                    stop=(l == GROUP_CHUNKS - 1),
                )

            u_sb = io_u.tile([128, 2, POOL_P], f32)
            nc.scalar.copy(out=u_sb[:, 0, :], in_=pool_a[:])
            nc.scalar.copy(out=u_sb[:, 1, :], in_=pool_b[:])
            s_sb = io_u.tile([POOL_P, 1], f32)
            nc.vector.tensor_copy(out=s_sb[:], in_=pool_s[:])
            nc.sync.dma_start(out=uT_d[g], in_=u_sb[:])
            nc.sync.dma_start(out=sS_d[g], in_=s_sb[:])

    nc.compile()
    return nc


def host_constants(Wu: np.ndarray, b: np.ndarray):
    Wu = np.asarray(Wu, np.float32)
    b = np.asarray(b, np.float32)
    wu_hi16 = Wu.astype(np.float16)
    wu_lo16 = (Wu - wu_hi16.astype(np.float32)).astype(np.float16)
    # [d, k] -> [d%128, d//128, k]
    wu_hi = np.ascontiguousarray(wu_hi16.reshape(4, 128, K).transpose(1, 0, 2))
    wu_lo = np.ascontiguousarray(wu_lo16.reshape(4, 128, K).transpose(1, 0, 2))
    brep = np.ascontiguousarray(np.broadcast_to(b, (128, K)))
    tp = np.arange(128)[:, None, None]
    ll = np.arange(GROUP_CHUNKS)[None, :, None]
    rr = np.arange(POOL_P)[None, None, :]
    masks = (((CHUNK * ll + tp) // N_TOK) == rr).astype(np.float32)
    ones = np.ones((128, 1), np.float32)
    return {"wu_hi": wu_hi, "wu_lo": wu_lo, "brep": brep, "masks": masks,
            "masks16": masks.astype(np.float16), "ones": ones}


def host_shard_inputs(m_shard: np.ndarray, p_shard: np.ndarray):
    """Per-shard data tensors: fp16 hi/lo feature-major chunk-blocked mpT
    and token-major fp16 m for the pooling stationary operand."""
    tokens = m_shard.shape[0] * N_TOK
    n_chunks = tokens // CHUNK
    x = np.concatenate(
        [m_shard.reshape(tokens, MD), p_shard.reshape(tokens, PD)], axis=1)
    xh = x.astype(np.float16)
    xl = (x - xh.astype(np.float32)).astype(np.float16)
    # [tok, 512] -> [128q, n_chunks, 4j, 128t]
    def to_fmajor(a):
        return a.reshape(n_chunks, CHUNK, 4, 128).transpose(3, 0, 2, 1)
    mpT = np.ascontiguousarray(
        np.stack([to_fmajor(xh), to_fmajor(xl)], axis=3))
    mhi = np.ascontiguousarray(xh[:, 0:MD])
    return {"mpT": mpT, "mhi": mhi}


def unshard_output(uT: np.ndarray, sS: np.ndarray) -> np.ndarray:
    """[n_groups,128,2,64] pooled sums + [n_groups,64,1] score sums ->
    normalized u [rows, 256]."""
    n_groups = uT.shape[0]
    # uT[g, q, h, r] -> u[g*64+r, h*128+q]
    u = uT.transpose(0, 3, 2, 1).reshape(n_groups * POOL_P, MD)
    S = sS.reshape(n_groups * POOL_P, 1)
    return u / S


_prog_cache: dict = {}


def get_program(b_shard: int):
    if b_shard not in _prog_cache:
        _prog_cache[b_shard] = build_program(b_shard)
    return _prog_cache[b_shard]


def kernel(m: np.ndarray, p: np.ndarray, Wu: np.ndarray, b: np.ndarray
           ) -> np.ndarray:
    m = np.ascontiguousarray(np.asarray(m, np.float32))
    p = np.ascontiguousarray(np.asarray(p, np.float32))
    B = m.shape[0]
    assert B % N_CORES == 0
    b_shard = B // N_CORES

    nc = get_program(b_shard)
    consts = host_constants(Wu, b)

    in_maps = []
    for c in range(N_CORES):
        ms = m[c * b_shard:(c + 1) * b_shard]
        ps = p[c * b_shard:(c + 1) * b_shard]
        in_maps.append({**host_shard_inputs(ms, ps), **consts})
    res = run_bass_kernel_spmd(nc, in_maps, list(range(N_CORES)))
    u = np.concatenate(
        [unshard_output(res.results[c]["uT"], res.results[c]["sS"])
         for c in range(N_CORES)], axis=0)
    return u.astype(np.float32)
